# revision 13
# baseline (speedup 1.0000x reference)
"""Trainium2 Bass kernel for nn_AttentionLayer (attention pooling over time).

Math (per sample b):
    logits[t] = u . tanh(X[b] @ W)[t]     # (T,)
    att       = softmax_t(logits)
    out[b]    = sum_t att[t] * X[b, t, :] # (D,)

Strategy:
  - Data-parallel over batch across 8 NeuronCores (B=64 -> 8 samples/core).
  - tanh bounds |logit| <= sum|u| < 5, so softmax needs NO max subtraction:
    p[t] = exp(logit[t]) is safe in fp32.  One streaming pass over X with
    PSUM accumulation of sum_t p[t]*x[t]; one divide per sample at the end.
  - The X@W matmul contracts over d, so it needs X^T (d on partitions); the
    weighted sum contracts over t, so it needs X natural (t on partitions).
    The host ships X as bf16 natural + fp8-e4m3 transposed (25.2 MiB/core
    total, the accuracy-feasible minimum: bass matmul has no int8 path and
    a single fp8 natural copy costs ~1.8e-2 rel err, at the gate).
  - At ~320-340 GB/s effective HBM rate the 25.2 MiB is ~75-79 us -- the
    DMA is the roofline.  The kernel therefore (a) keeps PE work below the
    DMA rate, (b) splits the slab streams over BOTH HWDGE queues so the
    xtt stream never queues behind the 2x-bigger xn stream (the measured
    mid-pipeline stalls were all xtt-sem waits), (c) fills the ~6 us DMA
    head with warm-up matmuls so HAM un-throttles the PE clock (4/8 ->
    8/8) before the real stream starts (~3.5 us measured cold penalty).
  - X@W runs as bf16 W (exact; the earlier fp8 hi+lo DoubleRow pair is
    matched on accuracy by construction and beaten on PE time: 2x213 ns
    vs 2x241 ns per supertile) x fp8 X^T moving, two K=128 halves
    accumulating in PSUM, each W-half LDWEIGHTS amortized over the
    supertile pair.  Numpy decomposition: W-quant error is negligible
    (4.076e-3 with exact W vs 4.064e-3 with hi+lo); dropping the lo plane
    instead would cost 1.38e-2 -- rejected.
  - logits (C stage): th [ctx, t] slices as stationary x u moving (N=1),
    4 per supertile; one [128, 8] PSUM group and ONE exp per supertile
    PAIR so the paps pool truly double-buffers; sum_t p on the idle
    Vector engine into per-sample scols.
  - weighted sum (E stage): p columns stationary x natural bf16 slabs
    moving (N=256), lagging the exp stage by 4 supertiles.
  - DMA queues: sync HWDGE = first-sample xtt halves (pipeline starts
    ~11 us instead of ~13), W, remaining xtt slabs, out stores; scalar
    HWDGE = u + all xn natural slabs.  gpsimd SWDGE is not used (its
    const loads measured landing 6 us late).
  - Measured history (shared host oscillates between clock regimes ~20%
    apart; compare adjacent runs only): previous best 90.2 us with the
    fp8 hi/lo DR A-stage, all slabs on one sync queue, no warmup.
"""

import numpy as np
import ml_dtypes

B, T, D, CTX = 64, 4096, 256, 100
NCORES = 8
BPC = B // NCORES          # samples per core
CP = 128                   # context dim padded to 128 (W/u zero-padded)
TSUP = 512                 # t-rows per supertile (one PSUM bank of xw)
BF16 = ml_dtypes.bfloat16
FP8 = ml_dtypes.float8_e4m3

_NC_CACHE: dict = {}


def build_nc(bpc=BPC, t_total=T):
    """Build (and cache) the Bass graph for one core's shard."""
    key = (bpc, t_total)
    if key in _NC_CACHE:
        return _NC_CACHE[key]

    from contextlib import ExitStack
    import concourse.bass as bass
    import concourse.tile as tile
    from concourse import bacc, mybir

    nsup = t_total // TSUP     # supertiles per sample (must be even)
    t_half = t_total // 2      # DMA slab = half a sample per layout
    nsup_h = nsup // 2         # supertiles per half-slab
    ns_h = t_half // 128       # t-rows per partition in one natural slab

    nc = bacc.Bacc("TRN2", target_bir_lowering=False, debug=False,
                   enable_asserts=False)
    x = nc.declare_dram_parameter("x", [bpc, t_total, D], mybir.dt.bfloat16,
                                  isOutput=False)
    xt = nc.declare_dram_parameter("xt", [bpc, D, 2, t_half],
                                   mybir.dt.float8e4, isOutput=False)
    w = nc.declare_dram_parameter("w", [D, CP], mybir.dt.bfloat16,
                                  isOutput=False)
    u = nc.declare_dram_parameter("u", [CP, 1], mybir.dt.bfloat16,
                                  isOutput=False)
    out = nc.declare_dram_parameter("out", [bpc, D], mybir.dt.float32,
                                    isOutput=True)

    FP32 = mybir.dt.float32
    BF = mybir.dt.bfloat16
    PSUM = bass.MemorySpace.PSUM
    AF = mybir.ActivationFunctionType

    with tile.TileContext(nc) as tc:
        with ExitStack() as ctx:
            const = ctx.enter_context(tc.tile_pool(name="const", bufs=1))
            xpool = ctx.enter_context(tc.tile_pool(name="x", bufs=8))
            xtpool = ctx.enter_context(tc.tile_pool(name="xt", bufs=2))
            thpool = ctx.enter_context(tc.tile_pool(name="th", bufs=4))
            ppool = ctx.enter_context(tc.tile_pool(name="p", bufs=12))
            fin = ctx.enter_context(tc.tile_pool(name="fin", bufs=4))
            xwps = ctx.enter_context(tc.tile_pool(name="xwps", bufs=2, space=PSUM))
            paps = ctx.enter_context(tc.tile_pool(name="paps", bufs=2, space=PSUM))
            oaps = ctx.enter_context(tc.tile_pool(name="oaps", bufs=2, space=PSUM))

            # State per sample, filled as the pipeline flows.
            xn = [None] * bpc
            xtt = [None] * bpc
            oacc = [None] * bpc      # [1, 260]: cols 0:256 out, 256 sum_p
            scols = [None] * bpc
            th = {}
            p_sb = {}

            # Head ordering on the sync queue: xtt0's h0 half (512 KiB,
            # feeds A pairs 0-1), then W (64 KiB), then the h1 half, then
            # the xn slab stream.  First A matmul gates on h0+W ~11 us.
            # xtt slabs for samples 1..7 ride the scalar HWDGE queue so
            # they never queue behind the 2x-bigger xn stream (v1 stall
            # mode) and their 8 issue slots barely load the ACT FIFO (the
            # v2 mistake was 16 xn issues there, which delayed xn behind
            # tanh/exp and stalled E every sample).
            xtt[0] = xtpool.tile([128, 2, 2, t_half], mybir.dt.float8e4,
                                 tag="xtt", name="xtt0")
            nc.sync.dma_start(
                xtt[0][:, :, 0, :],
                xt[0, :, 0, :].rearrange("(c p) t -> p c t", p=128))
            w_sb = const.tile([128, 2, CP], BF, tag="w")
            nc.sync.dma_start(w_sb[:], w.rearrange("(c p) m -> p c m", p=128))
            nc.sync.dma_start(
                xtt[0][:, :, 1, :],
                xt[0, :, 1, :].rearrange("(c p) t -> p c t", p=128))
            u_sb = const.tile([CP, 1], BF, tag="u")
            nc.scalar.dma_start(u_sb[:], u[:, :])
            onesf_sb = const.tile([128, 1], FP32, tag="onesf")
            nc.vector.memset(onesf_sb[:], 1.0)

            # Warm-up matmuls on a zeroed tile: keep the PE busy through
            # the DMA head so the HAM clock gate opens (4/8 -> 8/8 needs
            # ~3.4 us of sustained activity) before the first real matmul.
            wsrc = const.tile([128, 512], BF, tag="wsrc")
            nc.vector.memset(wsrc[:], 0.0)
            warm = xwps.tile([128, 2, TSUP], FP32, tag="xw", name="warm")
            for _ in range(11):
                nc.tensor.matmul(warm[:, 0, :], wsrc[:, 0:128], wsrc[:],
                                 start=True, stop=True)

            def supt(g):
                return divmod(g, nsup)  # -> (sample, supertile-in-sample)

            def stage_A(g):
                """xw matmul pair + tanh for supertiles g, g+1."""
                b, st = supt(g)
                def issue_xn(bb):
                    xn[bb] = [None, None]
                    for h in range(2):
                        xn[bb][h] = xpool.tile(
                            [128, ns_h, D], BF, tag="xn", name=f"xn{bb}_{h}")
                        nc.sync.dma_start(
                            xn[bb][h][:],
                            x[bb, h * t_half:(h + 1) * t_half,
                              :].rearrange("(p s) d -> p s d", p=128))

                if st == 0:
                    if b > 0:
                        # bufs=2 on xtpool is the prefetch throttle: only
                        # xtt1 can race the critical first-sample slabs;
                        # xtt[b+2] waits until xtt[b] is freed.  (An
                        # explicit gate copy on the Vector queue was
                        # measured to clog its strict FIFO and stall the
                        # scols/finalize chain for ~4 us.)
                        xtt[b] = xtpool.tile(
                            [128, 2, 2, t_half], mybir.dt.float8e4,
                            tag="xtt", name=f"xtt{b}")
                        nc.scalar.dma_start(
                            xtt[b][:],
                            xt[b].rearrange("(c p) h t -> p c h t", p=128))
                    issue_xn(b)
                    oacc[b] = oaps.tile([1, 260], FP32, tag="oacc",
                                        name=f"oacc{b}")
                    scols[b] = ppool.tile([128, nsup // 2], FP32,
                                          tag="scols", name=f"scols{b}")

                nq = 2
                # One 2-bank PSUM tile per pair; each supertile's matmuls
                # target their own bank (slice [:, i, :]), and ONE tanh
                # covers the pair ([128, 1024]): the ACT instruction's
                # ~352-cycle fixed cost is paid once, not twice (~9 us of
                # Scalar engine time across the kernel).
                xwp = xwps.tile([128, nq, TSUP], FP32, tag="xw",
                                name=f"xw{g}")
                # bf16 W (exact) x fp8 X^T moving, two K=128 halves
                # accumulating in PSUM; each W-half LDWEIGHTS serves both
                # supertiles of the pair.
                for c in range(2):
                    for i in range(nq):
                        sti = st + i
                        h = sti // nsup_h
                        j0 = (sti % nsup_h) * TSUP
                        nc.tensor.matmul(xwp[:, i, :],
                                         w_sb[:, c, :],
                                         xtt[b][:, c, h, j0:j0 + TSUP],
                                         start=(c == 0), stop=(c == 1))
                thp = thpool.tile([128, nq, TSUP], BF, tag="th",
                                  name=f"th{g}")
                nc.scalar.activation(thp[:], xwp[:], AF.Tanh)
                for i in range(nq):
                    th[g + i] = thp[:, i, :]

            def stage_C(g0):
                """logits + exp + (DVE) partial sum_p for the supertile
                pair (g0, g0+1).  One [128, 8] PSUM group and ONE exp per
                pair: paps gets true double-buffering (bufs=2 over one
                tile/iteration instead of two), so the next pair's logits
                matmuls never wait on the previous exp.
                """
                b, st0 = supt(g0)
                pcc = paps.tile([128, 8], FP32, tag="pacc",
                                name=f"pacc{g0}")
                for j in range(2):
                    g = g0 + j
                    for s in range(4):
                        nc.tensor.matmul(pcc[:, 4 * j + s:4 * j + s + 1],
                                         th[g][:, s * 128:(s + 1) * 128],
                                         u_sb[:],
                                         start=(j == 0 and s == 0),
                                         stop=(j == 1 and s == 3))
                    del th[g]
                pp = ppool.tile([128, 8], BF, tag="p", name=f"p{g0}")
                nc.scalar.activation(pp[:], pcc[:], AF.Exp)
                p_sb[g0 // 2] = pp
                nc.vector.reduce_sum(scols[b][:, st0 // 2:st0 // 2 + 1],
                                     pp[:], axis=mybir.AxisListType.X)

            def stage_E(g):
                """weighted-sum matmuls for supertile g (+ finalize)."""
                b, st = supt(g)
                pg, off = g // 2, (g % 2) * 4
                for s in range(4):
                    sg = 4 * st + s
                    h2, sl2 = sg // ns_h, sg % ns_h
                    nc.tensor.matmul(oacc[b][:, 0:D],
                                     p_sb[pg][:, off + s:off + s + 1],
                                     xn[b][h2][:, sl2, :],
                                     start=(sg == 0),
                                     stop=(sg == 4 * nsup - 1))
                if g % 2 == 1:
                    del p_sb[pg]
                if st == nsup - 1:
                    # Finalize sample b: out_row = oacc / sum_t p.  The
                    # scalar sum rides the spare PSUM columns of oacc.
                    s1v = fin.tile([128, 1], FP32, tag="s1v", name=f"s1v{b}")
                    nc.vector.reduce_sum(s1v[:], scols[b][:],
                                         axis=mybir.AxisListType.X)
                    nc.tensor.matmul(oacc[b][:, 256:257], onesf_sb[:],
                                     s1v[:])
                    rinv = fin.tile([1, 1], FP32, tag="rinv",
                                    name=f"rinv{b}")
                    nc.vector.reciprocal(rinv[:], oacc[b][:, 256:257])
                    osb = fin.tile([1, D], FP32, tag="osb", name=f"osb{b}")
                    nc.vector.tensor_scalar_mul(osb[:], oacc[b][:, 0:D],
                                                rinv[:])
                    nc.sync.dma_start(out[b:b + 1, :], osb[:])

            # Pair-wise software pipeline over all supertiles of all
            # samples.  Per pair-iteration: E for supertiles 2pi-4/2pi-3
            # (lag 4: never waits on exp), C/D for 2pi-2/2pi-1, A/B for
            # 2pi/2pi+1.  PE work is emitted ready-first (E, C, A).
            ntot = bpc * nsup
            npair = ntot // 2
            for pi in range(npair + 2):
                for gg in (2 * pi - 4, 2 * pi - 3):
                    if 0 <= gg < ntot:
                        stage_E(gg)
                if 0 <= 2 * pi - 2 < ntot:
                    stage_C(2 * pi - 2)
                if pi < npair:
                    stage_A(2 * pi)

    nc.compile()
    _NC_CACHE[key] = nc
    return nc


def make_in_maps(X, W, u, ncores=NCORES):
    """Shard + cast the full inputs for the cores.

    xt is stored t-permuted: column j = s*128 + p holds X[t = NS*p + s, :],
    matching the natural slab's partition layout (see build_nc docstring).
    """
    Xf = np.asarray(X)
    bpc = Xf.shape[0] // ncores
    t_total = Xf.shape[1]
    ns = t_total // 128
    Wp = np.zeros((D, CP), dtype=BF16)
    Wp[:, :CTX] = np.asarray(W, dtype=np.float32).astype(BF16)
    up = np.zeros((CP, 1), dtype=BF16)
    up[:CTX, :] = np.asarray(u).astype(BF16)
    X16 = Xf.astype(BF16)
    in_maps = []
    for i in range(ncores):
        xs = np.ascontiguousarray(X16[i * bpc:(i + 1) * bpc])
        # per half: [b, h, 128p, s, d] -> [b, h, d, s, p]; j = s*128 + p
        ns_h = ns // 2
        xs8 = Xf[i * bpc:(i + 1) * bpc].astype(FP8)
        xts = np.ascontiguousarray(
            xs8.reshape(bpc, 2, 128, ns_h, D).transpose(0, 4, 1, 3, 2)
        ).reshape(bpc, D, 2, t_total // 2)
        in_maps.append({"x": xs, "xt": xts, "w": Wp, "u": up})
    return in_maps


# test.py sets _PROFILE=True to capture neuron-profile exec time here.
_PROFILE = False
LAST_RESULT = None


def kernel(X, W, u):
    global LAST_RESULT
    from concourse.bass_utils import run_bass_kernel_spmd

    nc = build_nc()
    in_maps = make_in_maps(X, W, u)
    res = run_bass_kernel_spmd(nc, in_maps, core_ids=list(range(NCORES)),
                               trace=_PROFILE)
    LAST_RESULT = res
    outs = [np.asarray(res.results[i]["out"], dtype=np.float32)
            for i in range(NCORES)]
    return np.concatenate(outs, axis=0)


# revision 20
# speedup vs baseline: 1.0693x; 1.0693x over previous
"""Trainium2 Bass kernel for nn_AttentionLayer (attention pooling over time).

Math (per sample b):
    logits[t] = u . tanh(X[b] @ W)[t]     # (T,)
    att       = softmax_t(logits)
    out[b]    = sum_t att[t] * X[b, t, :] # (D,)

Strategy:
  - Data-parallel over batch across 8 NeuronCores (B=64 -> 8 samples/core).
  - tanh bounds |logit| <= sum|u| < 5, so softmax needs NO max subtraction:
    p[t] = exp(logit[t]) is safe in fp32.  One streaming pass over X with
    PSUM accumulation of sum_t p[t]*x[t]; one divide per sample at the end.
  - The X@W matmul contracts over d, so it needs X^T (d on partitions); the
    weighted sum contracts over t, so it needs X natural (t on partitions).
    The host ships X twice: fp8-e4m3 transposed (8.4 MiB/core) and a
    RESIDUAL-COMPENSATED mixed natural copy (12.6 MiB/core): odd t-chunks
    in fp8, even t-chunks in bf16 carrying the odd neighbour's fp8
    residual.  Attention weights are near-uniform (logit std ~0.12), so a
    residual landing on the adjacent timestep keeps ~5/6 of its
    correction: numpy-measured 5.7e-3 total rel err vs 1.65e-2 for naive
    half-fp8 (bass matmul has no int8 path; full-fp8 natural is 1.8e-2,
    at the gate).
  - At ~355 GB/s effective HBM rate the 21.1 MiB is ~60 us of DMA and the
    PE streams are ~60-62 us -- co-designed walls.  The kernel (a) splits
    the slab streams over BOTH HWDGE queues so the xtt stream never
    queues behind the 2x-bigger xn stream (measured mid-pipeline stalls
    were all xtt-sem waits), (b) fills the ~5 us DMA head with warm-up
    matmuls so HAM un-throttles the PE clock (4/8 -> 8/8) before the real
    stream starts (~3.5 us measured cold penalty).
  - X@W runs as bf16 W (exact; the earlier fp8 hi+lo DoubleRow pair is
    matched on accuracy by construction and beaten on PE time: 2x213 ns
    vs 2x241 ns per supertile) x fp8 X^T moving, two K=128 halves
    accumulating in PSUM, each W-half LDWEIGHTS amortized over the
    supertile pair.  Numpy decomposition: W-quant error is negligible
    (4.076e-3 with exact W vs 4.064e-3 with hi+lo); dropping the lo plane
    instead would cost 1.38e-2 -- rejected.
  - logits (C stage): th [ctx, t] slices as stationary x u moving (N=1),
    4 per supertile; one [128, 8] PSUM group and ONE exp per supertile
    PAIR so the paps pool truly double-buffers; sum_t p on the idle
    Vector engine into per-sample scols.
  - weighted sum (E stage): p columns stationary x natural bf16 slabs
    moving (N=256), lagging the exp stage by 4 supertiles.
  - DMA queues: sync HWDGE = first-sample xtt halves (pipeline starts
    ~11 us instead of ~13), W, remaining xtt slabs, out stores; scalar
    HWDGE = u + all xn natural slabs.  gpsimd SWDGE is not used (its
    const loads measured landing 6 us late).
  - Measured history (shared host oscillates between clock regimes ~20%
    apart; compare adjacent runs only): previous best 90.2 us with the
    fp8 hi/lo DR A-stage, all slabs on one sync queue, no warmup.
"""

import numpy as np
import ml_dtypes

B, T, D, CTX = 64, 4096, 256, 100
NCORES = 8
BPC = B // NCORES          # samples per core
CP = 128                   # context dim padded to 128 (W/u zero-padded)
TSUP = 512                 # t-rows per supertile (one PSUM bank of xw)
BF16 = ml_dtypes.bfloat16
FP8 = ml_dtypes.float8_e4m3

_NC_CACHE: dict = {}


def build_nc(bpc=BPC, t_total=T):
    """Build (and cache) the Bass graph for one core's shard."""
    key = (bpc, t_total)
    if key in _NC_CACHE:
        return _NC_CACHE[key]

    from contextlib import ExitStack
    import concourse.bass as bass
    import concourse.tile as tile
    from concourse import bacc, mybir

    nsup = t_total // TSUP     # supertiles per sample (must be even)
    t_half = t_total // 2      # DMA slab = half a sample per layout
    nsup_h = nsup // 2         # supertiles per half-slab
    ns_h = t_half // 128       # t-rows per partition in one natural slab

    nc = bacc.Bacc("TRN2", target_bir_lowering=False, debug=False,
                   enable_asserts=False)
    ns_h_ = (t_total // 2) // 128
    xb = nc.declare_dram_parameter("xb", [bpc, 2, 128, ns_h_ // 2, D],
                                   mybir.dt.bfloat16, isOutput=False)
    x8n = nc.declare_dram_parameter("x8n", [bpc, 2, 128, ns_h_ // 2, D],
                                    mybir.dt.float8e4, isOutput=False)
    xt = nc.declare_dram_parameter("xt", [bpc, D, 2, t_half],
                                   mybir.dt.float8e4, isOutput=False)
    w = nc.declare_dram_parameter("w", [D, CP], mybir.dt.bfloat16,
                                  isOutput=False)
    u = nc.declare_dram_parameter("u", [CP, 1], mybir.dt.bfloat16,
                                  isOutput=False)
    out = nc.declare_dram_parameter("out", [bpc, D], mybir.dt.float32,
                                    isOutput=True)

    FP32 = mybir.dt.float32
    BF = mybir.dt.bfloat16
    PSUM = bass.MemorySpace.PSUM
    AF = mybir.ActivationFunctionType

    with tile.TileContext(nc) as tc:
        with ExitStack() as ctx:
            const = ctx.enter_context(tc.tile_pool(name="const", bufs=1))
            xpool = ctx.enter_context(tc.tile_pool(name="x", bufs=16))
            xtpool = ctx.enter_context(tc.tile_pool(name="xt", bufs=2))
            thpool = ctx.enter_context(tc.tile_pool(name="th", bufs=4))
            ppool = ctx.enter_context(tc.tile_pool(name="p", bufs=12))
            fin = ctx.enter_context(tc.tile_pool(name="fin", bufs=4))
            xwps = ctx.enter_context(tc.tile_pool(name="xwps", bufs=2, space=PSUM))
            paps = ctx.enter_context(tc.tile_pool(name="paps", bufs=2, space=PSUM))
            oaps = ctx.enter_context(tc.tile_pool(name="oaps", bufs=2, space=PSUM))

            # State per sample, filled as the pipeline flows.
            xn = [None] * bpc
            xtt = [None] * bpc
            oacc = [None] * bpc      # [1, 260]: cols 0:256 out, 256 sum_p
            scols = [None] * bpc
            th = {}
            p_sb = {}

            # Head ordering on the sync queue: xtt0's h0 half (512 KiB,
            # feeds A pairs 0-1), then W (64 KiB), then the h1 half, then
            # the xn slab stream.  First A matmul gates on h0+W ~11 us.
            # xtt slabs for samples 1..7 ride the scalar HWDGE queue so
            # they never queue behind the 2x-bigger xn stream (v1 stall
            # mode) and their 8 issue slots barely load the ACT FIFO (the
            # v2 mistake was 16 xn issues there, which delayed xn behind
            # tanh/exp and stalled E every sample).
            xtt[0] = xtpool.tile([128, 2, 2, t_half], mybir.dt.float8e4,
                                 tag="xtt", name="xtt0")
            nc.sync.dma_start(
                xtt[0][:, :, 0, :],
                xt[0, :, 0, :].rearrange("(c p) t -> p c t", p=128))
            w_sb = const.tile([128, 2, CP], BF, tag="w")
            nc.sync.dma_start(w_sb[:], w.rearrange("(c p) m -> p c m", p=128))
            nc.sync.dma_start(
                xtt[0][:, :, 1, :],
                xt[0, :, 1, :].rearrange("(c p) t -> p c t", p=128))
            u_sb = const.tile([CP, 1], BF, tag="u")
            nc.scalar.dma_start(u_sb[:], u[:, :])
            onesf_sb = const.tile([128, 1], FP32, tag="onesf")
            nc.vector.memset(onesf_sb[:], 1.0)

            # Warm-up matmuls on a zeroed tile: keep the PE busy through
            # the DMA head so the HAM clock gate opens (4/8 -> 8/8 needs
            # ~3.4 us of sustained activity) before the first real matmul.
            wsrc = const.tile([128, 512], BF, tag="wsrc")
            nc.vector.memset(wsrc[:], 0.0)
            warm = xwps.tile([128, 2, TSUP], FP32, tag="xw", name="warm")
            for _ in range(11):
                nc.tensor.matmul(warm[:, 0, :], wsrc[:, 0:128], wsrc[:],
                                 start=True, stop=True)

            def supt(g):
                return divmod(g, nsup)  # -> (sample, supertile-in-sample)

            def stage_A(g):
                """xw matmul pair + tanh for supertiles g, g+1."""
                b, st = supt(g)
                def issue_xn(bb):
                    # Natural copy, f=1/2 residual-compensated mixed
                    # precision: even t-chunks ship bf16 (carrying the
                    # fp8 residual of their odd neighbour), odd chunks
                    # ship fp8.  12.6 MiB instead of 16.8 per core.
                    xn[bb] = [None, None]
                    for h in range(2):
                        tb = xpool.tile([128, ns_h // 2, D], BF,
                                        tag="xnb", name=f"xnb{bb}_{h}")
                        nc.sync.dma_start(tb[:], xb[bb, h])
                        t8 = xpool.tile([128, ns_h // 2, D],
                                        mybir.dt.float8e4,
                                        tag="xn8", name=f"xn8{bb}_{h}")
                        nc.sync.dma_start(t8[:], x8n[bb, h])
                        xn[bb][h] = (tb, t8)

                if st == 0:
                    if b > 0:
                        # bufs=2 on xtpool is the prefetch throttle: only
                        # xtt1 can race the critical first-sample slabs;
                        # xtt[b+2] waits until xtt[b] is freed.  (An
                        # explicit gate copy on the Vector queue was
                        # measured to clog its strict FIFO and stall the
                        # scols/finalize chain for ~4 us.)
                        xtt[b] = xtpool.tile(
                            [128, 2, 2, t_half], mybir.dt.float8e4,
                            tag="xtt", name=f"xtt{b}")
                        nc.scalar.dma_start(
                            xtt[b][:],
                            xt[b].rearrange("(c p) h t -> p c h t", p=128))
                    issue_xn(b)
                    oacc[b] = oaps.tile([1, 260], FP32, tag="oacc",
                                        name=f"oacc{b}")
                    scols[b] = ppool.tile([128, nsup // 2], FP32,
                                          tag="scols", name=f"scols{b}")

                nq = 2
                # One 2-bank PSUM tile per pair; each supertile's matmuls
                # target their own bank (slice [:, i, :]), and ONE tanh
                # covers the pair ([128, 1024]): the ACT instruction's
                # ~352-cycle fixed cost is paid once, not twice (~9 us of
                # Scalar engine time across the kernel).
                xwp = xwps.tile([128, nq, TSUP], FP32, tag="xw",
                                name=f"xw{g}")
                # bf16 W (exact) x fp8 X^T moving, two K=128 halves
                # accumulating in PSUM; each W-half LDWEIGHTS serves both
                # supertiles of the pair.
                for c in range(2):
                    for i in range(nq):
                        sti = st + i
                        h = sti // nsup_h
                        j0 = (sti % nsup_h) * TSUP
                        nc.tensor.matmul(xwp[:, i, :],
                                         w_sb[:, c, :],
                                         xtt[b][:, c, h, j0:j0 + TSUP],
                                         start=(c == 0), stop=(c == 1))
                # th in fp8: the C stage's per-logit LDWEIGHTS is its
                # wall-clock cost and FWL reads 4 fp8/cycle vs 2 bf16 --
                # halves the th load stream (~6.7 us of PE).  Accuracy
                # cost measured in numpy: 4.16e-3 -> 4.95e-3.
                thp = thpool.tile([128, nq, TSUP], mybir.dt.float8e4,
                                  tag="th", name=f"th{g}")
                nc.scalar.activation(thp[:], xwp[:], AF.Tanh)
                for i in range(nq):
                    th[g + i] = thp[:, i, :]

            def stage_C(g0):
                """logits + exp + (DVE) partial sum_p for the supertile
                pair (g0, g0+1).  One [128, 8] PSUM group and ONE exp per
                pair: paps gets true double-buffering (bufs=2 over one
                tile/iteration instead of two), so the next pair's logits
                matmuls never wait on the previous exp.
                """
                b, st0 = supt(g0)
                pcc = paps.tile([128, 8], FP32, tag="pacc",
                                name=f"pacc{g0}")
                for j in range(2):
                    g = g0 + j
                    for s in range(4):
                        nc.tensor.matmul(pcc[:, 4 * j + s:4 * j + s + 1],
                                         th[g][:, s * 128:(s + 1) * 128],
                                         u_sb[:],
                                         start=(j == 0 and s == 0),
                                         stop=(j == 1 and s == 3))
                    del th[g]
                pp = ppool.tile([128, 8], BF, tag="p", name=f"p{g0}")
                nc.scalar.activation(pp[:], pcc[:], AF.Exp)
                p_sb[g0 // 2] = pp
                nc.vector.reduce_sum(scols[b][:, st0 // 2:st0 // 2 + 1],
                                     pp[:], axis=mybir.AxisListType.X)

            def stage_E(g):
                """weighted-sum matmuls for supertile g (+ finalize)."""
                b, st = supt(g)
                pg, off = g // 2, (g % 2) * 4
                for s in range(4):
                    sg = 4 * st + s
                    h2, sl2 = sg // ns_h, sg % ns_h
                    rhs = xn[b][h2][sl2 % 2][:, sl2 // 2, :]
                    nc.tensor.matmul(oacc[b][:, 0:D],
                                     p_sb[pg][:, off + s:off + s + 1],
                                     rhs,
                                     start=(sg == 0),
                                     stop=(sg == 4 * nsup - 1))
                if g % 2 == 1:
                    del p_sb[pg]
                if st == nsup - 1:
                    # Finalize sample b: out_row = oacc / sum_t p.  The
                    # scalar sum rides the spare PSUM columns of oacc.
                    s1v = fin.tile([128, 1], FP32, tag="s1v", name=f"s1v{b}")
                    nc.vector.reduce_sum(s1v[:], scols[b][:],
                                         axis=mybir.AxisListType.X)
                    nc.tensor.matmul(oacc[b][:, 256:257], onesf_sb[:],
                                     s1v[:])
                    rinv = fin.tile([1, 1], FP32, tag="rinv",
                                    name=f"rinv{b}")
                    nc.vector.reciprocal(rinv[:], oacc[b][:, 256:257])
                    osb = fin.tile([1, D], FP32, tag="osb", name=f"osb{b}")
                    nc.vector.tensor_scalar_mul(osb[:], oacc[b][:, 0:D],
                                                rinv[:])
                    nc.sync.dma_start(out[b:b + 1, :], osb[:])

            # Pair-wise software pipeline over all supertiles of all
            # samples.  Per pair-iteration: E for supertiles 2pi-4/2pi-3
            # (lag 4: never waits on exp), C/D for 2pi-2/2pi-1, A/B for
            # 2pi/2pi+1.  PE work is emitted ready-first (E, C, A).
            ntot = bpc * nsup
            npair = ntot // 2
            for pi in range(npair + 2):
                for gg in (2 * pi - 4, 2 * pi - 3):
                    if 0 <= gg < ntot:
                        stage_E(gg)
                if 0 <= 2 * pi - 2 < ntot:
                    stage_C(2 * pi - 2)
                if pi < npair:
                    stage_A(2 * pi)

    nc.compile()
    _NC_CACHE[key] = nc
    return nc


def make_in_maps(X, W, u, ncores=NCORES):
    """Shard + cast the full inputs for the cores.

    xt is stored t-permuted: column j = s*128 + p holds X[t = NS*p + s, :],
    matching the natural slab's partition layout (see build_nc docstring).
    """
    Xf = np.asarray(X)
    bpc = Xf.shape[0] // ncores
    t_total = Xf.shape[1]
    ns = t_total // 128
    ns_h = ns // 2
    Wp = np.zeros((D, CP), dtype=BF16)
    Wp[:, :CTX] = np.asarray(W, dtype=np.float32).astype(BF16)
    up = np.zeros((CP, 1), dtype=BF16)
    up[:CTX, :] = np.asarray(u).astype(BF16)
    in_maps = []
    for i in range(ncores):
        Xc = Xf[i * bpc:(i + 1) * bpc]
        # natural layout [b, h, p, s, d]: t = h*t_half + p*ns_h + s
        X5 = Xc.reshape(bpc, 2, 128, ns_h, D)
        # f=1/2 residual compensation: odd s-chunks ship fp8; their
        # quantization residual is added to the even neighbour (adjacent
        # timestep, near-identical attention weight) shipped in bf16.
        q8 = X5[:, :, :, 1::2, :].astype(FP8)
        resid = X5[:, :, :, 1::2, :] - q8.astype(np.float32)
        xbf = np.ascontiguousarray(
            (X5[:, :, :, 0::2, :] + resid).astype(BF16))
        x8c = np.ascontiguousarray(q8)
        xs8 = Xc.astype(FP8)
        xts = np.ascontiguousarray(
            xs8.reshape(bpc, 2, 128, ns_h, D).transpose(0, 4, 1, 3, 2)
        ).reshape(bpc, D, 2, t_total // 2)
        in_maps.append({"xb": xbf, "x8n": x8c, "xt": xts, "w": Wp,
                        "u": up})
    return in_maps


# test.py sets _PROFILE=True to capture neuron-profile exec time here.
_PROFILE = False
LAST_RESULT = None


def kernel(X, W, u):
    global LAST_RESULT
    from concourse.bass_utils import run_bass_kernel_spmd

    nc = build_nc()
    in_maps = make_in_maps(X, W, u)
    res = run_bass_kernel_spmd(nc, in_maps, core_ids=list(range(NCORES)),
                               trace=_PROFILE)
    LAST_RESULT = res
    outs = [np.asarray(res.results[i]["out"], dtype=np.float32)
            for i in range(NCORES)]
    return np.concatenate(outs, axis=0)
